# revision 9
# baseline (speedup 1.0000x reference)
"""Cross-attention kernel for one TRN2 chip (8 NeuronCores).

Sharding: core = (batch b in {0,1}) x (head-group of 4 heads).  Each core
computes attention for its 4 heads of its batch element and a partial output
projection [N, 1024]; the host sums the 4 partials per batch and adds bias.

Key structure (all matmuls bf16, fp32 PSUM):
  - x/ctx pre-transposed on host, cast-loaded bf16; DMA ordered so the
    first K projection can start after ~1.5MB has landed.
  - QK per m-tile: two concurrent row-tiled matmuls (head s0 on array rows
    0-63, s1 on rows 64-127) into one [128,1024] PSUM tile; exp split
    across ScalarE (native, odd m) and DVE (int16 Schraudolph -> bf16 in
    ONE op, even m).
  - AV is v-stationary: lhsT = v[m,65] (with a ones column producing the
    softmax denominator), moving = pT[m,512].  Output accumulates in PSUM
    directly in d-major [d, n] layout (head0 partitions 0-64, head1
    63-127 with ones column FIRST so values align to partition 64) --
    no output transposes at all.
  - Normalize: DVE reciprocal of the denominator row, Pool-engine
    partition_broadcast, DVE tensor_mul into oTn (bf16).  Drain ops are
    emitted 2 m-tiles into the next group to avoid head-of-line stalls.
  - Output projection y = oTn.T @ wo per n-tile, streamed out per-tile.
"""

import numpy as np

import concourse.bass as bass
import concourse.mybir as mybir
import concourse.tile as tile
from concourse import bacc, library_config
from concourse.bass import ts
from concourse.bass_utils import run_bass_kernel_spmd

B, N, M, C = 2, 2048, 2048, 1024
HEADS, DH = 16, 64
H_PER = 4                # heads per core
DHC = H_PER * DH         # 256: per-core slice of INNER
SCALE = DH ** -0.5
P = 128
NT = N // P              # 16 n-tiles
MT = M // P              # 16 m-tiles
CCH = C // P             # 8 contraction chunks
FD = 512                 # attention n-chunk (PSUM bank)
NJ = N // FD             # 4 n-chunks
N_CORES = 8

F32 = mybir.dt.float32
BF16 = mybir.dt.bfloat16
I16 = mybir.dt.int16
EXP = mybir.ActivationFunctionType.Exp
# int16 Schraudolph fast-exp: exp(x*SCALE) ~= bitcast_bf16(i16(x*KS + BS))
KS16 = SCALE * (1 << 7) / float(np.log(2.0))
BS16 = float(127 * (1 << 7)) - 366392.0 / 65536.0

_CACHE = {}


def _build():
    nc = bacc.Bacc("TRN2", target_bir_lowering=False, debug=False,
                   num_devices=N_CORES, num_swdge_queues=4)

    xT_d = nc.dram_tensor("xT", (C, N), BF16, kind="ExternalInput").ap()
    cT_d = nc.dram_tensor("cT", (C, M), BF16, kind="ExternalInput").ap()
    msk_d = nc.dram_tensor("msk", (M, 1), F32, kind="ExternalInput").ap()
    wq_d = nc.dram_tensor("wq", (C, DHC), BF16, kind="ExternalInput").ap()
    wk_d = nc.dram_tensor("wk", (C, DHC), BF16, kind="ExternalInput").ap()
    wv_d = nc.dram_tensor("wv", (C, DHC), BF16, kind="ExternalInput").ap()
    wo_d = nc.dram_tensor("wo", (DHC, C), BF16, kind="ExternalInput").ap()
    y_d = nc.dram_tensor("y", (N, C), BF16, kind="ExternalOutput").ap()

    with tile.TileContext(nc) as tc:
        # partition_broadcast needs the attn gpsimd ucode library
        nc.gpsimd.load_library(library_config.attn)
        with (
            tc.tile_pool(name="const", bufs=1) as const,
            tc.tile_pool(name="pTp", bufs=4) as pTp,
            tc.tile_pool(name="drn", bufs=4) as drn,
            tc.tile_pool(name="rbp", bufs=4) as rbp,
            tc.tile_pool(name="yp", bufs=3) as yp,
        ):
            # ---- persistent SBUF tensors ----
            xT = const.tile([P, CCH, N], BF16, name="xT")
            cT = const.tile([P, CCH, M], BF16, name="cT")
            qT2 = const.tile([P, 2, N], BF16, name="qT2")
            kT2 = const.tile([P, 2, M], BF16, name="kT2")
            # v: [m-partition, m-tile, head, 65]; head s=0: [v(64) | ones],
            # head s=1: [ones | v(64)] so AV output aligns to partition 64.
            v_sb = const.tile([P, MT, H_PER, DH + 1], BF16, name="v")
            wq_sb = const.tile([P, CCH, DHC], BF16, name="wq")
            wk_sb = const.tile([P, CCH, DHC], BF16, name="wk")
            wv_sb = const.tile([P, CCH, DHC], BF16, name="wv")
            wo_sb = const.tile([P, 2, C], BF16, name="wo")
            msk_sb = const.tile([P, MT, 1], F32, name="msk")
            oTn = const.tile([P, 2, N], BF16, name="oTn")

            # ---- input DMA, ordered by first use ----
            nc.sync.dma_start(
                out=wk_sb, in_=wk_d.rearrange("(cc p) d -> p cc d", p=P))
            nc.sync.dma_start(
                out=msk_sb, in_=msk_d.rearrange("(t p) o -> p t o", p=P))
            cTv = cT_d.rearrange("(cc p) n -> p cc n", p=P)
            xTv = xT_d.rearrange("(cc p) n -> p cc n", p=P)
            for cc in range(CCH):
                nc.sync.dma_start(
                    out=cT[:, cc, ts(0, FD)], in_=cTv[:, cc, ts(0, FD)])
            nc.sync.dma_start(
                out=wv_sb, in_=wv_d.rearrange("(cc p) d -> p cc d", p=P))
            for g in range(1, 4):
                for cc in range(CCH):
                    nc.sync.dma_start(
                        out=cT[:, cc, ts(g, FD)], in_=cTv[:, cc, ts(g, FD)])
            nc.sync.dma_start(
                out=wq_sb, in_=wq_d.rearrange("(cc p) d -> p cc d", p=P))
            for g in range(4):
                for cc in range(CCH):
                    nc.sync.dma_start(
                        out=xT[:, cc, ts(g, FD)], in_=xTv[:, cc, ts(g, FD)])
            nc.sync.dma_start(
                out=wo_sb, in_=wo_d.rearrange("(dc p) e -> p dc e", p=P))

            nc.vector.memset(v_sb, 1.0)

            ps_p_cm = tc.tile_pool(name="ps_p", bufs=3, space="PSUM")
            ps_p = ps_p_cm.__enter__()       # [128,512] projections: 3 banks

            # project one n/m-chunk j of q or k (both d-chunks dc)
            def proj_T(w_sb, srcT, dstT2, dc, j, alt):
                ps = ps_p.tile([P, FD], F32, name="kq")
                for cc in range(CCH):
                    nc.tensor.matmul(
                        ps, lhsT=w_sb[:, cc, ts(dc, P)],
                        rhs=srcT[:, cc, ts(j, FD)],
                        start=(cc == 0), stop=(cc == CCH - 1))
                dst = dstT2[:, dc, ts(j, FD)]
                if alt:
                    nc.vector.tensor_copy(dst, ps)
                else:
                    nc.scalar.copy(dst, ps)

            # V projection for two m-tiles (one [128,512] PSUM tile)
            def proj_V(m0):
                vp = ps_p.tile([P, 2, DHC], F32, name="vp")
                for mi in range(2):
                    for cc in range(CCH):
                        nc.tensor.matmul(
                            vp[:, mi, :],
                            lhsT=cT[:, cc, ts(m0 + mi, P)],
                            rhs=wv_sb[:, cc, :],
                            start=(cc == 0), stop=(cc == CCH - 1))
                nc.vector.tensor_copy(
                    v_sb[:, m0:m0 + 2, :, 0:DH],
                    vp.rearrange("p mi (h d) -> p mi h d", h=H_PER))
                for mi in range(2):
                    nc.vector.tensor_scalar_mul(
                        v_sb[:, m0 + mi, :, :], v_sb[:, m0 + mi, :, :],
                        msk_sb[:, m0 + mi, :])

            # ---- phase A: K/V/Q projections ----
            alt = 0
            for g in range(4):
                for dc in range(2):
                    proj_T(wk_sb, cT, kT2, dc, g, alt % 2)
                    alt += 1
                proj_V(4 * g)
                proj_V(4 * g + 2)
            for g in range(4):
                for dc in range(2):
                    proj_T(wq_sb, xT, qT2, dc, g, alt % 2)
                    alt += 1

            ps_p_cm.__exit__(None, None, None)

            # ---- phase B: attention, AV v-stationary in d-major layout ----
            ps_s_cm = tc.tile_pool(name="ps_s", bufs=2, space="PSUM")
            ps_s = ps_s_cm.__enter__()       # [128,1024] scores: 4 banks
            ps_o_cm = tc.tile_pool(name="ps_o", bufs=2, space="PSUM")
            ps_o = ps_o_cm.__enter__()       # 2x[128,512] oT per grp: 4 banks

            def qk(sT, dc, j, m):
                for s in range(2):
                    nc.tensor.matmul(
                        sT[:, s, :],
                        lhsT=kT2[s * DH:(s + 1) * DH, dc, ts(m, P)],
                        rhs=qT2[s * DH:(s + 1) * DH, dc, ts(j, FD)],
                        start=True, stop=True)

            def av(oPs, pT, dc, m):
                # oT[d, n] += v[m, d|1].T @ pT[m, n]; stationary = v (65 col)
                for s in range(2):
                    nc.tensor.matmul(
                        oPs[s][0:DH + 1, :],
                        lhsT=v_sb[:, m, 2 * dc + s, :],
                        rhs=pT[:, s, :],
                        start=(m == 0), stop=(m == MT - 1),
                        skip_group_check=True)

            def drain(oPs, dc, j):
                # denominators at partition 64 of each head's PSUM tile
                rc = rbp.tile([P, 2, FD], F32, name="rc")
                nc.vector.reciprocal(rc[64:65, 0, :], oPs[0][64:65, :])
                nc.vector.reciprocal(rc[64:65, 1, :], oPs[1][64:65, :])
                # partition_broadcast reads partition 0 of its source
                # buffer, so hop the rc rows down to partition 0 by DMA
                rc0 = rbp.tile([P, 2, FD], F32, name="rc0")
                nc.sync.dma_start(out=rc0[0:1, :, :], in_=rc[64:65, :, :])
                rcb = rbp.tile([P, 2, FD], F32, name="rcb")
                nc.gpsimd.partition_broadcast(
                    rcb[0:DH, 0, :], rc0[0:1, 0, :], channels=DH)
                nc.gpsimd.partition_broadcast(
                    rcb[0:DH, 1, :], rc0[0:1, 1, :], channels=DH)
                nc.vector.tensor_mul(
                    oTn[0:DH, dc, ts(j, FD)], oPs[0][0:DH, :],
                    rcb[0:DH, 0, :])
                # head1 normalizes at partitions 0:64, then a small
                # SBUF->SBUF DMA shifts it to partitions 64:128 of oTn
                o1b = rbp.tile([P, FD], BF16, name="o1b")
                nc.vector.tensor_mul(
                    o1b[0:DH, :], oPs[1][0:DH, :], rcb[0:DH, 1, :])
                nc.sync.dma_start(
                    out=oTn[DH:P, dc, ts(j, FD)], in_=o1b[0:DH, :])

            pend = None
            for j in range(NJ):
                for dc in range(2):
                    oPs = [ps_o.tile([P, FD], F32, name=f"o{s}")
                           for s in range(2)]
                    pTs = []
                    for m in range(MT):
                        sT = ps_s.tile([P, 2, FD], F32, name="sT")
                        qk(sT, dc, j, m)
                        if m == 3 and pend is not None:
                            drain(*pend)
                            pend = None
                        if m >= 2:
                            av(oPs, pTs[m - 2], dc, m - 2)
                        pT = pTp.tile([P, 2, FD], BF16, name="pT")
                        if m % 2 == 0:
                            # DVE int16 Schraudolph -> bf16 in one op
                            nc.vector.tensor_scalar(
                                pT.bitcast(I16)[:, :, :], sT, KS16, BS16,
                                op0=mybir.AluOpType.mult,
                                op1=mybir.AluOpType.add)
                        else:
                            nc.scalar.activation(pT, sT, EXP, scale=SCALE)
                        pTs.append(pT)
                    av(oPs, pTs[MT - 2], dc, MT - 2)
                    av(oPs, pTs[MT - 1], dc, MT - 1)
                    pend = (oPs, dc, j)
            drain(*pend)

            ps_o_cm.__exit__(None, None, None)
            ps_s_cm.__exit__(None, None, None)

            # ---- phase C: output projection ----
            ps_y_cm = tc.tile_pool(name="ps_y", bufs=3, space="PSUM")
            ps_y = ps_y_cm.__enter__()

            def y_tile(i):
                y_ps = ps_y.tile([P, C], F32, name="y")
                for col in range(2):
                    for dc in range(2):
                        nc.tensor.matmul(
                            y_ps[:, ts(col, FD)],
                            lhsT=oTn[:, dc, ts(i, P)],
                            rhs=wo_sb[:, dc, ts(col, FD)],
                            start=(dc == 0), stop=(dc == 1))
                y_sb = yp.tile([P, C], BF16, name="ysb")
                nc.vector.tensor_copy(y_sb[:, 0:FD], y_ps[:, 0:FD])
                nc.scalar.copy(y_sb[:, FD:C], y_ps[:, FD:C])
                nc.sync.dma_start(out=y_d[ts(i, P), :], in_=y_sb)

            for i in range(NT):
                y_tile(i)
            ps_y_cm.__exit__(None, None, None)

    nc.compile()
    return nc


def _in_maps(x, context, mask, Wq, Wk, Wv, Wo):
    from ml_dtypes import bfloat16
    maps = []
    xb = np.asarray(x, dtype=np.float32).astype(bfloat16)
    cb = np.asarray(context, dtype=np.float32).astype(bfloat16)
    for core in range(N_CORES):
        b, hg = core // H_PER, core % H_PER
        c0 = hg * DHC
        maps.append({
            "xT": np.ascontiguousarray(xb[b].T),
            "cT": np.ascontiguousarray(cb[b].T),
            "msk": np.ascontiguousarray(
                np.asarray(mask[b]).astype(np.float32).reshape(M, 1)),
            "wq": np.ascontiguousarray(
                np.asarray(Wq[:, c0:c0 + DHC], dtype=np.float32)
                .astype(bfloat16)),
            "wk": np.ascontiguousarray(
                np.asarray(Wk[:, c0:c0 + DHC], dtype=np.float32)
                .astype(bfloat16)),
            "wv": np.ascontiguousarray(
                np.asarray(Wv[:, c0:c0 + DHC], dtype=np.float32)
                .astype(bfloat16)),
            "wo": np.ascontiguousarray(
                np.asarray(Wo[c0:c0 + DHC, :], dtype=np.float32)
                .astype(bfloat16)),
        })
    return maps


def _gather(results, bo):
    out = np.zeros((B, N, C), dtype=np.float32)
    for core in range(N_CORES):
        out[core // H_PER] += np.asarray(results[core]["y"],
                                         dtype=np.float32)
    out += np.asarray(bo, dtype=np.float32)
    return out


def kernel(x, context, mask, Wq, Wk, Wv, Wo, bo, **extra_kwargs):
    if "nc" not in _CACHE:
        _CACHE["nc"] = _build()
    nc = _CACHE["nc"]
    maps = _in_maps(x, context, mask, Wq, Wk, Wv, Wo)
    res = run_bass_kernel_spmd(nc, maps, core_ids=list(range(N_CORES)),
                               **extra_kwargs)
    out = _gather(res.results, bo)
    if extra_kwargs:
        _CACHE["last_result"] = res
    return out


# revision 12
# speedup vs baseline: 1.3065x; 1.3065x over previous
"""Cross-attention kernel for one TRN2 chip (8 NeuronCores).

Sharding: core = (batch b in {0,1}) x (head-group of 4 heads).  Each core
computes attention for its 4 heads of its batch element and a partial output
projection [N, 1024]; the host sums the 4 partials per batch and adds bias.

Key structure (all matmuls bf16, fp32 PSUM):
  - x/ctx pre-transposed on host, cast-loaded bf16; DMA ordered so the
    first K projection can start after ~1.5MB has landed.
  - QK per m-tile: two concurrent row-tiled matmuls into one [128,1024]
    PSUM tile from a 3-deep pool so the PE runs 3 tiles ahead of exp and
    never drops out of the 2.4GHz p-state.
  - exp is split across THREE engines per 16-tile group: ScalarE native
    exp (9), DVE int16-Schraudolph->bf16 in one op (5), Pool(GpSimd)
    int16-Schraudolph (2).  AV consumes pT at lag 4 so even the slow
    Pool exp is ready in time.
  - AV is v-stationary: lhsT = v[m, d|ones], moving = pT[m,512]; output
    accumulates in PSUM in d-major [d, n] layout (no output transposes),
    with the ones column producing the softmax denominator in row 64.
  - Drain: two DVE copies PSUM->SBUF free the oT banks within ~1.5us;
    the normalization (reshape-DMA -> [128,8] reciprocal -> DRAM-bounce
    partition-broadcast -> two DVE mults -> head1 partition-shift DMA)
    is fully deferred off the critical path.
  - Output projection y = oTn.T @ wo per n-tile, streamed out per-tile.
"""

import dataclasses

import numpy as np

import concourse.bass as bass
import concourse.mybir as mybir
import concourse.tile as tile
from concourse import bacc
from concourse.bass import ts
from concourse.bass_utils import run_bass_kernel_spmd

B, N, M, C = 2, 2048, 2048, 1024
HEADS, DH = 16, 64
H_PER = 4                # heads per core
DHC = H_PER * DH         # 256: per-core slice of INNER
SCALE = DH ** -0.5
P = 128
NT = N // P              # 16 n-tiles
MT = M // P              # 16 m-tiles
CCH = C // P             # 8 contraction chunks
FD = 512                 # attention n-chunk (PSUM bank)
NJ = N // FD             # 4 n-chunks
N_CORES = 8
LAG = 4                  # av trails qk/exp by this many m-tiles

F32 = mybir.dt.float32
BF16 = mybir.dt.bfloat16
I16 = mybir.dt.int16
EXP = mybir.ActivationFunctionType.Exp
# int16 Schraudolph fast-exp: exp(x*SCALE) ~= bitcast_bf16(i16(x*KS + BS))
KS16 = SCALE * (1 << 7) / float(np.log(2.0))
BS16 = float(127 * (1 << 7)) - 366392.0 / 65536.0

# exp engine assignment per m-tile within a group: odd -> DVE
ENG_DVE = {1, 3, 5, 7, 9, 11, 13, 15}

_CACHE = {}


def _build():
    nc = bacc.Bacc("TRN2", target_bir_lowering=False, debug=False,
                   num_devices=N_CORES, num_swdge_queues=4)

    xT_d = nc.dram_tensor("xT", (C, N), BF16, kind="ExternalInput").ap()
    cT_d = nc.dram_tensor("cT", (C, M), BF16, kind="ExternalInput").ap()
    msk_d = nc.dram_tensor("msk", (M, 1), F32, kind="ExternalInput").ap()
    wq_d = nc.dram_tensor("wq", (C, DHC), BF16, kind="ExternalInput").ap()
    wk_d = nc.dram_tensor("wk", (C, DHC), BF16, kind="ExternalInput").ap()
    wv_d = nc.dram_tensor("wv", (C, DHC), BF16, kind="ExternalInput").ap()
    wo_d = nc.dram_tensor("wo", (DHC, C), BF16, kind="ExternalInput").ap()
    y_d = nc.dram_tensor("y", (N, C), BF16, kind="ExternalOutput").ap()

    with tile.TileContext(nc) as tc:
        with (
            tc.tile_pool(name="const", bufs=1) as const,
            tc.tile_pool(name="pTp", bufs=6) as pTp,
            tc.tile_pool(name="orp", bufs=3) as orp,
            tc.tile_pool(name="rbp", bufs=2) as rbp,
            tc.tile_pool(name="yp", bufs=3) as yp,
            tc.tile_pool(name="dramp", bufs=8, space="DRAM") as dramp,
        ):
            # ---- persistent SBUF tensors ----
            xT = const.tile([P, CCH, N], BF16, name="xT")
            cT = const.tile([P, CCH, M], BF16, name="cT")
            qT2 = const.tile([P, 2, N], BF16, name="qT2")
            kT2 = const.tile([P, 2, M], BF16, name="kT2")
            # v: [m-partition, m-tile, head, d(64)+ones(1)]
            v_sb = const.tile([P, MT, H_PER, DH + 1], BF16, name="v")
            wq_sb = const.tile([P, CCH, DHC], BF16, name="wq")
            wk_sb = const.tile([P, CCH, DHC], BF16, name="wk")
            wv_sb = const.tile([P, CCH, DHC], BF16, name="wv")
            wo_sb = const.tile([P, 2, C], BF16, name="wo")
            msk_sb = const.tile([P, MT, 1], F32, name="msk")
            oTn = const.tile([P, 2, N], BF16, name="oTn")

            # ---- input DMA, ordered by first use ----
            nc.sync.dma_start(
                out=wk_sb, in_=wk_d.rearrange("(cc p) d -> p cc d", p=P))
            nc.sync.dma_start(
                out=msk_sb, in_=msk_d.rearrange("(t p) o -> p t o", p=P))
            cTv = cT_d.rearrange("(cc p) n -> p cc n", p=P)
            xTv = xT_d.rearrange("(cc p) n -> p cc n", p=P)
            for cc in range(CCH):
                nc.sync.dma_start(
                    out=cT[:, cc, ts(0, FD)], in_=cTv[:, cc, ts(0, FD)])
            nc.sync.dma_start(
                out=wv_sb, in_=wv_d.rearrange("(cc p) d -> p cc d", p=P))
            for g in range(1, 4):
                for cc in range(CCH):
                    nc.sync.dma_start(
                        out=cT[:, cc, ts(g, FD)], in_=cTv[:, cc, ts(g, FD)])
            nc.sync.dma_start(
                out=wq_sb, in_=wq_d.rearrange("(cc p) d -> p cc d", p=P))
            for g in range(4):
                for cc in range(CCH):
                    nc.sync.dma_start(
                        out=xT[:, cc, ts(g, FD)], in_=xTv[:, cc, ts(g, FD)])
            nc.sync.dma_start(
                out=wo_sb, in_=wo_d.rearrange("(dc p) e -> p dc e", p=P))

            nc.vector.memset(v_sb, 1.0)

            # shared PSUM pool: projections (phase A) + scores (phase B)
            ps_cm = tc.tile_pool(name="ps", bufs=3, space="PSUM")
            ps = ps_cm.__enter__()           # 3 x 2 banks

            def proj_T(w_sb, srcT, dstT2, dc, j, alt):
                pt = ps.tile([P, 2, FD], F32, name="ps")
                for cc in range(CCH):
                    nc.tensor.matmul(
                        pt[:, 0, :], lhsT=w_sb[:, cc, ts(dc, P)],
                        rhs=srcT[:, cc, ts(j, FD)],
                        start=(cc == 0), stop=(cc == CCH - 1))
                dst = dstT2[:, dc, ts(j, FD)]
                if alt:
                    nc.vector.tensor_copy(dst, pt[:, 0, :])
                else:
                    nc.scalar.copy(dst, pt[:, 0, :])

            def proj_V(m0):
                vp = ps.tile([P, 2, FD], F32, name="ps")
                vv = vp.rearrange("p mi (h d) -> p mi h d", h=H_PER * 2)
                for mi in range(2):
                    for cc in range(CCH):
                        nc.tensor.matmul(
                            vp[:, mi, 0:DHC],
                            lhsT=cT[:, cc, ts(m0 + mi, P)],
                            rhs=wv_sb[:, cc, :],
                            start=(cc == 0), stop=(cc == CCH - 1))
                nc.vector.tensor_copy(
                    v_sb[:, m0:m0 + 2, :, 0:DH],
                    vp[:, :, 0:DHC].rearrange("p mi (h d) -> p mi h d",
                                              h=H_PER))
                del vv
                for mi in range(2):
                    nc.vector.tensor_scalar_mul(
                        v_sb[:, m0 + mi, :, :], v_sb[:, m0 + mi, :, :],
                        msk_sb[:, m0 + mi, :])

            # ---- phase A: K/V/Q projections ----
            alt = 0
            for g in range(4):
                for dc in range(2):
                    proj_T(wk_sb, cT, kT2, dc, g, alt % 2)
                    alt += 1
                proj_V(4 * g)
                proj_V(4 * g + 2)
            for g in range(4):
                for dc in range(2):
                    proj_T(wq_sb, xT, qT2, dc, g, alt % 2)
                    alt += 1

            # ---- phase B: attention ----
            ps_o_cm = tc.tile_pool(name="ps_o", bufs=1, space="PSUM")
            ps_o = ps_o_cm.__enter__()       # 2 x 1 bank oT accumulators

            def qk(sT, dc, j, m):
                for s in range(2):
                    nc.tensor.matmul(
                        sT[:, s, :],
                        lhsT=kT2[s * DH:(s + 1) * DH, dc, ts(m, P)],
                        rhs=qT2[s * DH:(s + 1) * DH, dc, ts(j, FD)],
                        start=True, stop=True)

            def av(oPs, pT, dc, m):
                # oT[d, n] += v[m, d|1].T @ pT[m, n]; stationary = v (65 col)
                for s in range(2):
                    nc.tensor.matmul(
                        oPs[s][0:DH + 1, :],
                        lhsT=v_sb[:, m, 2 * dc + s, :],
                        rhs=pT[:, s, :],
                        start=(m == 0), stop=(m == MT - 1),
                        skip_group_check=True)

            def normalize(o_raw, dc, j):
                # denominators live in o_raw row 64 as [1, 2, 512]:
                # reshape-DMA to [128, 8], fast reciprocal, bounce through
                # DRAM, broadcast-read to 64 partitions, multiply.
                rden = rbp.tile([P, 8], F32, name="rden")
                src = o_raw[DH:DH + 1, :, :]
                src_r = dataclasses.replace(
                    src, ap=[src.ap[0], (8, P), (1, 8)])
                nc.sync.dma_start(out=rden, in_=src_r)
                rrec = rbp.tile([P, 8], F32, name="rrec")
                nc.vector.reciprocal(rrec, rden)
                scr = dramp.tile([2, FD], F32, name="scr")
                nc.sync.dma_start(out=scr, in_=rrec)
                rcb = rbp.tile([P, 2, FD], F32, name="rcb")
                sap = scr[:, :]
                bap = dataclasses.replace(
                    sap, ap=[(0, DH)] + list(sap.ap))
                nc.sync.dma_start(out=rcb[0:DH, :, :], in_=bap)
                nc.gpsimd.tensor_mul(
                    oTn[0:DH, dc, ts(j, FD)], o_raw[0:DH, 0, :],
                    rcb[0:DH, 0, :])
                o1b = rbp.tile([P, FD], BF16, name="o1b")
                nc.gpsimd.tensor_mul(
                    o1b[0:DH, :], o_raw[0:DH, 1, :], rcb[0:DH, 1, :])
                nc.sync.dma_start(
                    out=oTn[DH:P, dc, ts(j, FD)], in_=o1b[0:DH, :])

            for j in range(NJ):
                for dc in range(2):
                    oPs = [ps_o.tile([P, FD], F32, name=f"o{s}")
                           for s in range(2)]
                    pTs = []
                    for m in range(MT):
                        sT = ps.tile([P, 2, FD], F32, name="ps")
                        qk(sT, dc, j, m)
                        if m >= LAG:
                            av(oPs, pTs[m - LAG], dc, m - LAG)
                        pT = pTp.tile([P, 2, FD], BF16, name="pT")
                        if m in ENG_DVE:
                            nc.vector.tensor_scalar(
                                pT.bitcast(I16)[:, :, :], sT, KS16, BS16,
                                op0=mybir.AluOpType.mult,
                                op1=mybir.AluOpType.add)
                        else:
                            nc.scalar.activation(pT, sT, EXP, scale=SCALE)
                        pTs.append(pT)
                    for t in range(LAG):
                        av(oPs, pTs[MT - LAG + t], dc, MT - LAG + t)
                    # drain: free the oT banks fast, normalize deferred
                    o_raw = orp.tile([P, 2, FD], F32, name="o_raw")
                    nc.scalar.copy(
                        o_raw[0:DH + 1, 0, :], oPs[0][0:DH + 1, :])
                    nc.scalar.copy(
                        o_raw[0:DH + 1, 1, :], oPs[1][0:DH + 1, :])
                    normalize(o_raw, dc, j)

            ps_o_cm.__exit__(None, None, None)
            ps_cm.__exit__(None, None, None)

            # ---- phase C: output projection ----
            ps_y_cm = tc.tile_pool(name="ps_y", bufs=3, space="PSUM")
            ps_y = ps_y_cm.__enter__()

            def y_tile(i):
                y_ps = ps_y.tile([P, C], F32, name="y")
                for col in range(2):
                    for dc in range(2):
                        nc.tensor.matmul(
                            y_ps[:, ts(col, FD)],
                            lhsT=oTn[:, dc, ts(i, P)],
                            rhs=wo_sb[:, dc, ts(col, FD)],
                            start=(dc == 0), stop=(dc == 1))
                y_sb = yp.tile([P, C], BF16, name="ysb")
                nc.vector.tensor_copy(y_sb[:, 0:FD], y_ps[:, 0:FD])
                nc.scalar.copy(y_sb[:, FD:C], y_ps[:, FD:C])
                nc.sync.dma_start(out=y_d[ts(i, P), :], in_=y_sb)

            for i in range(NT):
                y_tile(i)
            ps_y_cm.__exit__(None, None, None)

    nc.compile()
    return nc


def _in_maps(x, context, mask, Wq, Wk, Wv, Wo):
    from ml_dtypes import bfloat16
    maps = []
    xb = np.asarray(x, dtype=np.float32).astype(bfloat16)
    cb = np.asarray(context, dtype=np.float32).astype(bfloat16)
    for core in range(N_CORES):
        b, hg = core // H_PER, core % H_PER
        c0 = hg * DHC
        maps.append({
            "xT": np.ascontiguousarray(xb[b].T),
            "cT": np.ascontiguousarray(cb[b].T),
            "msk": np.ascontiguousarray(
                np.asarray(mask[b]).astype(np.float32).reshape(M, 1)),
            "wq": np.ascontiguousarray(
                np.asarray(Wq[:, c0:c0 + DHC], dtype=np.float32)
                .astype(bfloat16)),
            "wk": np.ascontiguousarray(
                np.asarray(Wk[:, c0:c0 + DHC], dtype=np.float32)
                .astype(bfloat16)),
            "wv": np.ascontiguousarray(
                np.asarray(Wv[:, c0:c0 + DHC], dtype=np.float32)
                .astype(bfloat16)),
            "wo": np.ascontiguousarray(
                np.asarray(Wo[c0:c0 + DHC, :], dtype=np.float32)
                .astype(bfloat16)),
        })
    return maps


def _gather(results, bo):
    out = np.zeros((B, N, C), dtype=np.float32)
    for core in range(N_CORES):
        out[core // H_PER] += np.asarray(results[core]["y"],
                                         dtype=np.float32)
    out += np.asarray(bo, dtype=np.float32)
    return out


def kernel(x, context, mask, Wq, Wk, Wv, Wo, bo, **extra_kwargs):
    if "nc" not in _CACHE:
        _CACHE["nc"] = _build()
    nc = _CACHE["nc"]
    maps = _in_maps(x, context, mask, Wq, Wk, Wv, Wo)
    res = run_bass_kernel_spmd(nc, maps, core_ids=list(range(N_CORES)),
                               **extra_kwargs)
    out = _gather(res.results, bo)
    if extra_kwargs:
        _CACHE["last_result"] = res
    return out


# revision 13
# speedup vs baseline: 1.3168x; 1.0079x over previous
"""Cross-attention kernel for one TRN2 chip (8 NeuronCores).

Sharding: core = (batch b in {0,1}) x (head-group of 4 heads).  Each core
computes attention for its 4 heads of its batch element and a partial output
projection [N, 1024]; the host sums the 4 partials per batch and adds bias.

Key structure (all matmuls bf16, fp32 PSUM):
  - x/ctx pre-transposed on host, cast-loaded bf16; DMA ordered so the
    first K projection can start after ~1.5MB has landed.
  - QK per m-tile: two concurrent row-tiled matmuls into one [128,1024]
    PSUM tile from a 3-deep pool so the PE runs 3 tiles ahead of exp and
    never drops out of the 2.4GHz p-state.
  - exp is split across THREE engines per 16-tile group: ScalarE native
    exp (9), DVE int16-Schraudolph->bf16 in one op (5), Pool(GpSimd)
    int16-Schraudolph (2).  AV consumes pT at lag 4 so even the slow
    Pool exp is ready in time.
  - AV is v-stationary: lhsT = v[m, d|ones], moving = pT[m,512]; output
    accumulates in PSUM in d-major [d, n] layout (no output transposes),
    with the ones column producing the softmax denominator in row 64.
  - Drain: two DVE copies PSUM->SBUF free the oT banks within ~1.5us;
    the normalization (reshape-DMA -> [128,8] reciprocal -> DRAM-bounce
    partition-broadcast -> two DVE mults -> head1 partition-shift DMA)
    is fully deferred off the critical path.
  - Output projection y = oTn.T @ wo per n-tile, streamed out per-tile.
"""

import dataclasses

import numpy as np

import concourse.bass as bass
import concourse.mybir as mybir
import concourse.tile as tile
from concourse import bacc
from concourse.bass import ts
from concourse.bass_utils import run_bass_kernel_spmd

B, N, M, C = 2, 2048, 2048, 1024
HEADS, DH = 16, 64
H_PER = 4                # heads per core
DHC = H_PER * DH         # 256: per-core slice of INNER
SCALE = DH ** -0.5
P = 128
NT = N // P              # 16 n-tiles
MT = M // P              # 16 m-tiles
CCH = C // P             # 8 contraction chunks
FD = 512                 # attention n-chunk (PSUM bank)
NJ = N // FD             # 4 n-chunks
N_CORES = 8
LAG = 5                  # av trails qk/exp by this many m-tiles

F32 = mybir.dt.float32
BF16 = mybir.dt.bfloat16
I16 = mybir.dt.int16
EXP = mybir.ActivationFunctionType.Exp
# int16 Schraudolph fast-exp: exp(x*SCALE) ~= bitcast_bf16(i16(x*KS + BS))
KS16 = SCALE * (1 << 7) / float(np.log(2.0))
BS16 = float(127 * (1 << 7)) - 366392.0 / 65536.0

# exp engine assignment per m-tile within a group: odd -> DVE
ENG_DVE = {1, 3, 5, 7, 9, 11, 13, 15}

_CACHE = {}


def _build():
    nc = bacc.Bacc("TRN2", target_bir_lowering=False, debug=False,
                   num_devices=N_CORES, num_swdge_queues=4)

    xT_d = nc.dram_tensor("xT", (C, N), BF16, kind="ExternalInput").ap()
    cT_d = nc.dram_tensor("cT", (C, M), BF16, kind="ExternalInput").ap()
    msk_d = nc.dram_tensor("msk", (M, 1), F32, kind="ExternalInput").ap()
    wq_d = nc.dram_tensor("wq", (C, DHC), BF16, kind="ExternalInput").ap()
    wk_d = nc.dram_tensor("wk", (C, DHC), BF16, kind="ExternalInput").ap()
    wv_d = nc.dram_tensor("wv", (C, DHC), BF16, kind="ExternalInput").ap()
    wo_d = nc.dram_tensor("wo", (DHC, C), BF16, kind="ExternalInput").ap()
    y_d = nc.dram_tensor("y", (N, C), BF16, kind="ExternalOutput").ap()

    with tile.TileContext(nc) as tc:
        with (
            tc.tile_pool(name="const", bufs=1) as const,
            tc.tile_pool(name="pTp", bufs=7) as pTp,
            tc.tile_pool(name="orp", bufs=3) as orp,
            tc.tile_pool(name="rbp", bufs=2) as rbp,
            tc.tile_pool(name="yp", bufs=3) as yp,
            tc.tile_pool(name="dramp", bufs=8, space="DRAM") as dramp,
        ):
            # ---- persistent SBUF tensors ----
            xT = const.tile([P, CCH, N], BF16, name="xT")
            cT = const.tile([P, CCH, M], BF16, name="cT")
            qT2 = const.tile([P, 2, N], BF16, name="qT2")
            kT2 = const.tile([P, 2, M], BF16, name="kT2")
            # v: [m-partition, m-tile, head, d(64)+ones(1)]
            v_sb = const.tile([P, MT, H_PER, DH + 1], BF16, name="v")
            wq_sb = const.tile([P, CCH, DHC], BF16, name="wq")
            wk_sb = const.tile([P, CCH, DHC], BF16, name="wk")
            wv_sb = const.tile([P, CCH, DHC], BF16, name="wv")
            wo_sb = const.tile([P, 2, C], BF16, name="wo")
            msk_sb = const.tile([P, MT, 1], F32, name="msk")
            oTn = const.tile([P, 2, N], BF16, name="oTn")

            # ---- input DMA, ordered by first use ----
            nc.sync.dma_start(
                out=wk_sb, in_=wk_d.rearrange("(cc p) d -> p cc d", p=P))
            nc.sync.dma_start(
                out=msk_sb, in_=msk_d.rearrange("(t p) o -> p t o", p=P))
            cTv = cT_d.rearrange("(cc p) n -> p cc n", p=P)
            xTv = xT_d.rearrange("(cc p) n -> p cc n", p=P)
            nc.sync.dma_start(
                out=cT[:, :, ts(0, FD)], in_=cTv[:, :, ts(0, FD)])
            nc.sync.dma_start(
                out=wv_sb, in_=wv_d.rearrange("(cc p) d -> p cc d", p=P))
            for g in range(1, 4):
                nc.sync.dma_start(
                    out=cT[:, :, ts(g, FD)], in_=cTv[:, :, ts(g, FD)])
            nc.sync.dma_start(
                out=wq_sb, in_=wq_d.rearrange("(cc p) d -> p cc d", p=P))
            for g in range(4):
                nc.sync.dma_start(
                    out=xT[:, :, ts(g, FD)], in_=xTv[:, :, ts(g, FD)])
            nc.sync.dma_start(
                out=wo_sb, in_=wo_d.rearrange("(dc p) e -> p dc e", p=P))

            nc.vector.memset(v_sb, 1.0)

            # shared PSUM pool: projections (phase A) + scores (phase B)
            ps_cm = tc.tile_pool(name="ps", bufs=3, space="PSUM")
            ps = ps_cm.__enter__()           # 3 x 2 banks

            def proj_T(w_sb, srcT, dstT2, dc, j, alt):
                pt = ps.tile([P, 2, FD], F32, name="ps")
                for cc in range(CCH):
                    nc.tensor.matmul(
                        pt[:, 0, :], lhsT=w_sb[:, cc, ts(dc, P)],
                        rhs=srcT[:, cc, ts(j, FD)],
                        start=(cc == 0), stop=(cc == CCH - 1))
                dst = dstT2[:, dc, ts(j, FD)]
                if alt:
                    nc.vector.tensor_copy(dst, pt[:, 0, :])
                else:
                    nc.scalar.copy(dst, pt[:, 0, :])

            def proj_V(m0):
                vp = ps.tile([P, 2, FD], F32, name="ps")
                vv = vp.rearrange("p mi (h d) -> p mi h d", h=H_PER * 2)
                for mi in range(2):
                    for cc in range(CCH):
                        nc.tensor.matmul(
                            vp[:, mi, 0:DHC],
                            lhsT=cT[:, cc, ts(m0 + mi, P)],
                            rhs=wv_sb[:, cc, :],
                            start=(cc == 0), stop=(cc == CCH - 1))
                nc.vector.tensor_copy(
                    v_sb[:, m0:m0 + 2, :, 0:DH],
                    vp[:, :, 0:DHC].rearrange("p mi (h d) -> p mi h d",
                                              h=H_PER))
                del vv
                for mi in range(2):
                    nc.vector.tensor_scalar_mul(
                        v_sb[:, m0 + mi, :, :], v_sb[:, m0 + mi, :, :],
                        msk_sb[:, m0 + mi, :])

            # ---- phase A: K/V/Q projections ----
            alt = 0
            for g in range(4):
                for dc in range(2):
                    proj_T(wk_sb, cT, kT2, dc, g, alt % 2)
                    alt += 1
                proj_V(4 * g)
                proj_V(4 * g + 2)
            for g in range(4):
                for dc in range(2):
                    proj_T(wq_sb, xT, qT2, dc, g, alt % 2)
                    alt += 1

            # ---- phase B: attention ----
            ps_o_cm = tc.tile_pool(name="ps_o", bufs=1, space="PSUM")
            ps_o = ps_o_cm.__enter__()       # 2 x 1 bank oT accumulators

            def qk(sT, dc, j, m):
                for s in range(2):
                    nc.tensor.matmul(
                        sT[:, s, :],
                        lhsT=kT2[s * DH:(s + 1) * DH, dc, ts(m, P)],
                        rhs=qT2[s * DH:(s + 1) * DH, dc, ts(j, FD)],
                        start=True, stop=True)

            def av(oPs, pT, dc, m):
                # oT[d, n] += v[m, d|1].T @ pT[m, n]; stationary = v (65 col)
                for s in range(2):
                    nc.tensor.matmul(
                        oPs[s][0:DH + 1, :],
                        lhsT=v_sb[:, m, 2 * dc + s, :],
                        rhs=pT[:, s, :],
                        start=(m == 0), stop=(m == MT - 1),
                        skip_group_check=True)

            def normalize(o_raw, dc, j):
                # denominators live in o_raw row 64 as [1, 2, 512]:
                # reshape-DMA to [128, 8], fast reciprocal, bounce through
                # DRAM, broadcast-read to 64 partitions, multiply.
                rden = rbp.tile([P, 8], F32, name="rden")
                src = o_raw[DH:DH + 1, :, :]
                src_r = dataclasses.replace(
                    src, ap=[src.ap[0], (8, P), (1, 8)])
                nc.gpsimd.dma_start(out=rden, in_=src_r)
                rrec = rbp.tile([P, 8], F32, name="rrec")
                nc.vector.reciprocal(rrec, rden)
                scr = dramp.tile([2, FD], F32, name="scr")
                nc.gpsimd.dma_start(out=scr, in_=rrec)
                rcb = rbp.tile([P, 2, FD], F32, name="rcb")
                sap = scr[:, :]
                bap = dataclasses.replace(
                    sap, ap=[(0, DH)] + list(sap.ap))
                nc.gpsimd.dma_start(out=rcb[0:DH, :, :], in_=bap)
                nc.gpsimd.tensor_mul(
                    oTn[0:DH, dc, ts(j, FD)], o_raw[0:DH, 0, :],
                    rcb[0:DH, 0, :])
                o1b = rbp.tile([P, FD], BF16, name="o1b")
                nc.gpsimd.tensor_mul(
                    o1b[0:DH, :], o_raw[0:DH, 1, :], rcb[0:DH, 1, :])
                nc.gpsimd.dma_start(
                    out=oTn[DH:P, dc, ts(j, FD)], in_=o1b[0:DH, :])

            def drain(oPs, dc, j):
                o_raw = orp.tile([P, 2, FD], F32, name="o_raw")
                nc.scalar.copy(
                    o_raw[0:DH + 1, 0, :], oPs[0][0:DH + 1, :])
                nc.scalar.copy(
                    o_raw[0:DH + 1, 1, :], oPs[1][0:DH + 1, :])
                normalize(o_raw, dc, j)

            pend = None
            for j in range(NJ):
                for dc in range(2):
                    oPs = [ps_o.tile([P, FD], F32, name=f"o{s}")
                           for s in range(2)]
                    pTs = []
                    for m in range(MT):
                        sT = ps.tile([P, 2, FD], F32, name="ps")
                        qk(sT, dc, j, m)
                        if m == 1 and pend is not None:
                            drain(*pend)
                            pend = None
                        if m >= LAG:
                            av(oPs, pTs[m - LAG], dc, m - LAG)
                        pT = pTp.tile([P, 2, FD], BF16, name="pT")
                        if m in ENG_DVE:
                            nc.vector.tensor_scalar(
                                pT.bitcast(I16)[:, :, :], sT, KS16, BS16,
                                op0=mybir.AluOpType.mult,
                                op1=mybir.AluOpType.add)
                        else:
                            nc.scalar.activation(pT, sT, EXP, scale=SCALE)
                        pTs.append(pT)
                    for t in range(LAG):
                        av(oPs, pTs[MT - LAG + t], dc, MT - LAG + t)
                    pend = (oPs, dc, j)
            drain(*pend)

            ps_o_cm.__exit__(None, None, None)
            ps_cm.__exit__(None, None, None)

            # ---- phase C: output projection ----
            ps_y_cm = tc.tile_pool(name="ps_y", bufs=3, space="PSUM")
            ps_y = ps_y_cm.__enter__()

            def y_tile(i):
                y_ps = ps_y.tile([P, C], F32, name="y")
                for col in range(2):
                    for dc in range(2):
                        nc.tensor.matmul(
                            y_ps[:, ts(col, FD)],
                            lhsT=oTn[:, dc, ts(i, P)],
                            rhs=wo_sb[:, dc, ts(col, FD)],
                            start=(dc == 0), stop=(dc == 1))
                y_sb = yp.tile([P, C], BF16, name="ysb")
                nc.vector.tensor_copy(y_sb[:, 0:FD], y_ps[:, 0:FD])
                nc.scalar.copy(y_sb[:, FD:C], y_ps[:, FD:C])
                nc.sync.dma_start(out=y_d[ts(i, P), :], in_=y_sb)

            for i in range(NT):
                y_tile(i)
            ps_y_cm.__exit__(None, None, None)

    nc.compile()
    return nc


def _in_maps(x, context, mask, Wq, Wk, Wv, Wo):
    from ml_dtypes import bfloat16
    maps = []
    xb = np.asarray(x, dtype=np.float32).astype(bfloat16)
    cb = np.asarray(context, dtype=np.float32).astype(bfloat16)
    for core in range(N_CORES):
        b, hg = core // H_PER, core % H_PER
        c0 = hg * DHC
        maps.append({
            "xT": np.ascontiguousarray(xb[b].T),
            "cT": np.ascontiguousarray(cb[b].T),
            "msk": np.ascontiguousarray(
                np.asarray(mask[b]).astype(np.float32).reshape(M, 1)),
            "wq": np.ascontiguousarray(
                np.asarray(Wq[:, c0:c0 + DHC], dtype=np.float32)
                .astype(bfloat16)),
            "wk": np.ascontiguousarray(
                np.asarray(Wk[:, c0:c0 + DHC], dtype=np.float32)
                .astype(bfloat16)),
            "wv": np.ascontiguousarray(
                np.asarray(Wv[:, c0:c0 + DHC], dtype=np.float32)
                .astype(bfloat16)),
            "wo": np.ascontiguousarray(
                np.asarray(Wo[c0:c0 + DHC, :], dtype=np.float32)
                .astype(bfloat16)),
        })
    return maps


def _gather(results, bo):
    out = np.zeros((B, N, C), dtype=np.float32)
    for core in range(N_CORES):
        out[core // H_PER] += np.asarray(results[core]["y"],
                                         dtype=np.float32)
    out += np.asarray(bo, dtype=np.float32)
    return out


def kernel(x, context, mask, Wq, Wk, Wv, Wo, bo, **extra_kwargs):
    if "nc" not in _CACHE:
        _CACHE["nc"] = _build()
    nc = _CACHE["nc"]
    maps = _in_maps(x, context, mask, Wq, Wk, Wv, Wo)
    res = run_bass_kernel_spmd(nc, maps, core_ids=list(range(N_CORES)),
                               **extra_kwargs)
    out = _gather(res.results, bo)
    if extra_kwargs:
        _CACHE["last_result"] = res
    return out
